# revision 20
# baseline (speedup 1.0000x reference)
"""Trainium2 Bass kernel for nn_ActorCritic (PointNet conv trunk + LSTM actor-critic).

Self-contained: hardcodes all shapes/sharding. Strategy:
  - Shard the N=262144 lidar points across 8 NeuronCores (32768 each).
  - Per core: conv trunk 2->64->128->256 as f32r PE matmuls over 512-col
    chunks (layer-1 packs 2 point-groups into K=4/M=128), relu+bias on ACT,
    per-chunk max-pool on DVE straight out of PSUM (bias+relu folded after
    pooling - both are monotonic per-channel).
  - AllGather the per-core (256,) channel maxes, reduce across cores locally.
  - Replicated tail on every core: fc -> pose-net concat -> pw -> LSTMCell ->
    actor softmax + twin critics. Matvecs keep M on partitions (N=1 matmuls)
    so the LSTM elementwise runs at [128,4] instead of [1,512].
  - All small weights travel in one packed [128, NCOL] tensor (2 DMAs) since
    per-dma_start fixed overhead (~0.65us) dominates small transfers.
  - Core 0's outputs are returned.
"""

import numpy as np

import concourse.bass as bass
import concourse.mybir as mybir
import concourse.tile as tile
from concourse import bacc

N_CORES = 8
N_POINTS = 262144
NSH = N_POINTS // N_CORES          # 32768 points per core
F = 512                            # free-dim chunk (psum bank)
G2 = NSH // (2 * F)                # 32 double-chunks (2 groups of 512 pts)

f32 = mybir.dt.float32
f32r = mybir.dt.float32r
AF = mybir.ActivationFunctionType
ALU = mybir.AluOpType
AX = mybir.AxisListType

# Packed small-weight layout: name -> (partition_rows, cols). Conv weights
# first so the first (small) DMA unblocks the conv trunk immediately.
_PACK_CONV = [
    ("c1wg", 4, 128), ("c1b2", 128, 1), ("c2wT", 128, 128), ("c2b", 128, 1),
    ("c3wT", 128, 256), ("c3b2", 128, 2),
]
_PACK_TAIL = [
    ("fcwT", 128, 2 * 256), ("fcb2", 128, 2),
    ("odo", 3, 1), ("ow1T", 3, 16), ("ob1", 16, 1),
    ("ow2T", 16, 32), ("ob2", 32, 1),
    ("pwT", 128, 3 * 512), ("pb4", 128, 4), ("bsum", 128, 16),
    ("hxr", 128, 4), ("cxr", 128, 4),
    ("q1T", 128, 4 * 128), ("q1b", 128, 1), ("q2T", 128, 64), ("q2b", 64, 1),
    ("aw1T", 64, 16), ("ab1", 16, 1), ("aw2T", 16, 5), ("ab2", 1, 5),
    ("iw1T", 64, 16), ("ib1", 16, 1), ("iw2T", 16, 1), ("ib2", 1, 1),
    ("ew1T", 64, 16), ("eb1", 16, 1), ("ew2T", 16, 1), ("eb2", 1, 1),
]
_PACK = _PACK_CONV + _PACK_TAIL
_COL_OFF = {}
_c = 0
for _nm, _p, _w in _PACK:
    _COL_OFF[_nm] = (_c, _p, _w)
    _c += _w
NCOL = _c
CONV_COLS = sum(w for _, _, w in _PACK_CONV)

_INPUT_DECLS = {
    "lidar4": ((4, NSH // 2), f32r),
    "wpack": ((128, NCOL), f32r),
    "wihT": ((4, 128, 2048), f32),
    "whhT": ((4, 128, 2048), f32),
}

_OUTPUT_DECLS = {
    "o_actor": (5,),
    "o_cic": (2,),
    "o_hx": (512,),
    "o_cx": (512,),
    "o_ni": (288,),
}


class _WSlices:
    """Weight APs sliced out of the packed SBUF tile."""

    def __init__(self, wpack_sb):
        self._t = wpack_sb

    def __getitem__(self, name):
        c0, p, wd = _COL_OFF[name]
        ap = self._t[0:p, c0:c0 + wd]
        if name == "fcwT":
            return ap.rearrange("p (k n) -> p k n", k=2)
        if name == "pwT":
            return ap.rearrange("p (k n) -> p k n", k=3)
        if name == "q1T":
            return ap.rearrange("p (k n) -> p k n", k=4)
        return ap


def build(reps: int = 1, sim_mode: bool = False):
    nc = bacc.Bacc("TRN2", target_bir_lowering=False, debug=False,
                   num_devices=1 if sim_mode else N_CORES)
    din = {name: nc.dram_tensor(name, list(shape), dt, kind="ExternalInput").ap()
           for name, (shape, dt) in _INPUT_DECLS.items()}
    dout = {name: nc.dram_tensor(name, list(shape), f32, kind="ExternalOutput").ap()
            for name, shape in _OUTPUT_DECLS.items()}

    with tile.TileContext(nc) as tc:
        with tc.tile_pool(name="wsb", bufs=1) as wsb, \
             tc.tile_pool(name="csb", bufs=1) as csb, \
             tc.tile_pool(name="tsb", bufs=1) as tsb, \
             tc.tile_pool(name="pconv", bufs=1, space="PSUM") as pconv, \
             tc.tile_pool(name="ptail", bufs=1, space="PSUM") as ptail, \
             tc.tile_pool(name="dpool", bufs=1, space="DRAM") as dpool:

            actwarm = wsb.tile([1, 1], f32, name="actwarm")
            nc.vector.memset(actwarm[:], 0.0)
            actwarm2 = wsb.tile([1, 1], f32, name="actwarm2")
            nc.scalar.activation(actwarm2[:], actwarm[:], AF.Sigmoid)

            wpack_sb = wsb.tile([128, NCOL], f32r, name="wpack_sb")
            # conv weights first (small DMA), rest of the small weights second
            nc.sync.dma_start(out=wpack_sb[:, 0:CONV_COLS],
                              in_=din["wpack"][:, 0:CONV_COLS])
            w = _WSlices(wpack_sb)

            big = {}
            for name in ("wihT", "whhT"):
                big[name] = wsb.tile([128, 4, 2048], f32, name=f"w_{name}")

            def load_big_weights():
                nc.sync.dma_start(out=wpack_sb[:, CONV_COLS:NCOL],
                                  in_=din["wpack"][:, CONV_COLS:NCOL])
                for name in ("wihT", "whhT"):
                    t = big[name]
                    ns = 2048 // 4
                    for j in range(4):
                        nc.sync.dma_start(
                            out=t[:, :, j * ns:(j + 1) * ns],
                            in_=din[name][:, :, j * ns:(j + 1) * ns]
                            .rearrange("k p n -> p k n"))

            for i in range(reps):
                _one_rep(nc, tc, din, dout, w, big, csb, tsb, pconv, ptail,
                         dpool, sim_mode=sim_mode,
                         post_lidar=load_big_weights if i == 0 else None)

    nc.compile()
    return nc


def _one_rep(nc, tc, din, dout, w, big, csb, tsb, pconv, ptail, dpool,
             sim_mode=False, post_lidar=None):
    # ================= conv trunk over point chunks =======================
    lidar_sb = csb.tile([4, NSH // 2], f32r, tag="lidar", bufs=1)
    nblk = 4
    bs = (NSH // 2) // nblk
    for j in range(nblk):
        nc.sync.dma_start(out=lidar_sb[:, j * bs:(j + 1) * bs],
                          in_=din["lidar4"][:, j * bs:(j + 1) * bs])
    if post_lidar is not None:
        post_lidar()

    cm0 = csb.tile([128, G2], f32, tag="cm0", bufs=2)
    cm1 = csb.tile([128, G2], f32, tag="cm1", bufs=2)

    for g in range(G2):
        gs = slice(g * F, (g + 1) * F)
        p1 = pconv.tile([128, F], f32, tag="p1", bufs=1)
        nc.tensor.matmul(p1[:], w["c1wg"], lidar_sb[:, gs], start=True, stop=True)
        h1 = csb.tile([128, F], f32r, tag="h1", bufs=3)
        nc.scalar.activation(h1[:], p1[:], AF.Relu, bias=w["c1b2"].bitcast(f32))

        p2a = pconv.tile([128, F], f32, tag="p2", bufs=2)
        nc.tensor.matmul(p2a[:], w["c2wT"][0:64, :], h1[0:64, :], start=True, stop=True)
        p2b = pconv.tile([128, F], f32, tag="p2", bufs=2)
        nc.tensor.matmul(p2b[:], w["c2wT"][64:128, :], h1[64:128, :], start=True, stop=True)
        h2 = csb.tile([128, 2 * F], f32r, tag="h2", bufs=2)
        nc.scalar.activation(h2[:, 0:F], p2a[:], AF.Relu, bias=w["c2b"].bitcast(f32))
        nc.scalar.activation(h2[:, F:2 * F], p2b[:], AF.Relu, bias=w["c2b"].bitcast(f32))
        h2a, h2b = h2[:, 0:F], h2[:, F:2 * F]

        # both groups' L3 outputs for one m-tile share a [128, 1024] psum
        # (2 banks) so the max-pool is a single DVE reduce per m-tile.
        p3m0 = pconv.tile([128, 2 * F], f32, tag="p3", bufs=2)
        nc.tensor.matmul(p3m0[:, 0:F], w["c3wT"][:, 0:128], h2a, start=True, stop=True)
        nc.tensor.matmul(p3m0[:, F:2 * F], w["c3wT"][:, 0:128], h2b, start=True, stop=True)
        nc.vector.reduce_max(cm0[:, g:g + 1], p3m0[:], axis=AX.X)
        p3m1 = pconv.tile([128, 2 * F], f32, tag="p3", bufs=2)
        nc.tensor.matmul(p3m1[:, 0:F], w["c3wT"][:, 128:256], h2a, start=True, stop=True)
        nc.tensor.matmul(p3m1[:, F:2 * F], w["c3wT"][:, 128:256], h2b, start=True, stop=True)
        nc.vector.reduce_max(cm1[:, g:g + 1], p3m1[:], axis=AX.X)

        if g == 20:
            # LSTM whh-part: fills PE bubbles while conv runs; fold to SBUF
            # immediately so the psum bank frees for the tail chain.
            psum_gh = ptail.tile([128, 16], f32, tag="ptail", bufs=1)
            for j in range(16):
                js = slice(j * 128, (j + 1) * 128)
                for k in range(4):
                    nc.tensor.matmul(psum_gh[:, j:j + 1], big["whhT"][:, k, js],
                                     w["hxr"].bitcast(f32)[:, k:k + 1],
                                     start=(k == 0), stop=(k == 3))
            whhb = tsb.tile([128, 16], f32, tag="whhb", bufs=2)
            nc.vector.tensor_tensor(whhb[:], psum_gh[:], w["bsum"].bitcast(f32),
                                    ALU.add)

    praw = tsb.tile([128, 2], f32, tag="praw", bufs=2)
    nc.vector.reduce_max(praw[:, 0:1], cm0[:], axis=AX.X)
    nc.vector.reduce_max(praw[:, 1:2], cm1[:], axis=AX.X)

    # ================= AllGather of per-core channel maxes ================
    ag_in = dpool.tile([256], f32, tag="ag_in", bufs=2)
    ag_out = dpool.tile([N_CORES * 256], f32, tag="ag_out", bufs=2,
                        addr_space="Local" if sim_mode else "Shared")
    nc.sync.dma_start(out=ag_in[:].rearrange("(t p) -> p t", p=128), in_=praw[:])
    if sim_mode:
        for r in range(N_CORES):
            nc.sync.dma_start(out=ag_out[r * 256:(r + 1) * 256], in_=ag_in[:])
    else:
        nc.gpsimd.collective_compute(
            "AllGather", ALU.bypass,
            replica_groups=[list(range(N_CORES))],
            ins=[ag_in[:].opt()], outs=[ag_out[:].opt()])
    agt = tsb.tile([128, 16], f32, tag="agt", bufs=2)
    nc.sync.dma_start(out=agt[:],
                      in_=ag_out[:].rearrange("(r t p) -> p r t", p=128, t=2))
    pm = tsb.tile([128, 2], f32, tag="pm", bufs=2)
    nc.vector.reduce_max(pm[:], agt[:].rearrange("p (r t) -> p t r", r=8, t=2),
                         axis=AX.X)
    hmax = tsb.tile([128, 2], f32, tag="hmax", bufs=2)
    for t in range(2):
        nc.scalar.activation(hmax[:, t:t + 1], pm[:, t:t + 1], AF.Relu,
                             bias=w["c3b2"].bitcast(f32)[:, t:t + 1])

    # ================= fc: obs = fcw @ pooled + fcb (no relu) =============
    psum_obs = ptail.tile([128, 2], f32, tag="ptail", bufs=1)
    for m in range(2):
        ms = slice(m * 128, (m + 1) * 128)
        for k in range(2):
            nc.tensor.matmul(psum_obs[:, m:m + 1],
                             w["fcwT"].bitcast(f32)[:, k, ms],
                             hmax[:, k:k + 1], start=(k == 0), stop=(k == 1))
    ni01 = tsb.tile([128, 2], f32, tag="ni01", bufs=2)
    for m in range(2):
        nc.scalar.activation(ni01[:, m:m + 1], psum_obs[:, m:m + 1], AF.Identity,
                             bias=w["fcb2"].bitcast(f32)[:, m:m + 1])
    nc.sync.dma_start(out=dout["o_ni"][0:256].rearrange("(m p) -> p m", p=128),
                      in_=ni01[:])

    # ================= pose net ==========================================
    pp1 = ptail.tile([16, 1], f32, tag="ptail", bufs=1)
    nc.tensor.matmul(pp1[:], w["ow1T"].bitcast(f32), w["odo"].bitcast(f32),
                     start=True, stop=True)
    p1s = tsb.tile([16, 1], f32, tag="p1s", bufs=2)
    nc.scalar.activation(p1s[:], pp1[:], AF.Sigmoid, bias=w["ob1"].bitcast(f32))
    pp2 = ptail.tile([32, 1], f32, tag="ptail", bufs=1)
    nc.tensor.matmul(pp2[:], w["ow2T"].bitcast(f32), p1s[:], start=True, stop=True)
    ni2 = tsb.tile([32, 1], f32, tag="ni2", bufs=2)
    nc.scalar.activation(ni2[:], pp2[:], AF.Relu, bias=w["ob2"].bitcast(f32))
    nc.sync.dma_start(out=dout["o_ni"][256:288], in_=ni2[:])

    # ================= pw: z = relu(pw @ net_input + pb) ==================
    psum_z = ptail.tile([128, 4], f32, tag="ptail", bufs=1)
    pwT = w["pwT"].bitcast(f32)
    for m in range(4):
        ms = slice(m * 128, (m + 1) * 128)
        nc.tensor.matmul(psum_z[:, m:m + 1], pwT[:, 0, ms], ni01[:, 0:1],
                         start=True, stop=False)
        nc.tensor.matmul(psum_z[:, m:m + 1], pwT[:, 1, ms], ni01[:, 1:2],
                         start=False, stop=False)
        nc.tensor.matmul(psum_z[:, m:m + 1], pwT[0:32, 2, ms], ni2[:],
                         start=False, stop=True)
    z_sb = tsb.tile([128, 4], f32, tag="z_sb", bufs=2)
    for m in range(4):
        nc.scalar.activation(z_sb[:, m:m + 1], psum_z[:, m:m + 1], AF.Relu,
                             bias=w["pb4"].bitcast(f32)[:, m:m + 1])

    # ================= LSTM cell =========================================
    psum_g = ptail.tile([128, 16], f32, tag="ptail", bufs=1)
    for j in range(16):
        js = slice(j * 128, (j + 1) * 128)
        for k in range(4):
            nc.tensor.matmul(psum_g[:, j:j + 1], big["wihT"][:, k, js],
                             z_sb[:, k:k + 1], start=(k == 0), stop=(k == 3))
    gates = tsb.tile([128, 16], f32, tag="gates", bufs=2)
    nc.vector.tensor_tensor(gates[:], psum_g[:], whhb[:], ALU.add)
    gi = tsb.tile([128, 4], f32, tag="gi", bufs=2)
    gf = tsb.tile([128, 4], f32, tag="gf", bufs=2)
    gg = tsb.tile([128, 4], f32, tag="gg", bufs=2)
    go = tsb.tile([128, 4], f32, tag="go", bufs=2)
    nc.scalar.activation(gi[:], gates[:, 0:4], AF.Sigmoid)
    nc.scalar.activation(gf[:], gates[:, 4:8], AF.Sigmoid)
    nc.scalar.activation(gg[:], gates[:, 8:12], AF.Tanh)
    nc.scalar.activation(go[:], gates[:, 12:16], AF.Sigmoid)
    t1 = tsb.tile([128, 4], f32, tag="t1", bufs=2)
    nc.vector.tensor_tensor(t1[:], gi[:], gg[:], ALU.mult)
    t2 = tsb.tile([128, 4], f32, tag="t2", bufs=2)
    nc.vector.tensor_tensor(t2[:], gf[:], w["cxr"].bitcast(f32), ALU.mult)
    cxn = tsb.tile([128, 4], f32, tag="cxn", bufs=2)
    nc.vector.tensor_tensor(cxn[:], t1[:], t2[:], ALU.add)
    nc.sync.dma_start(out=dout["o_cx"][:].rearrange("(j p) -> p j", p=128),
                      in_=cxn[:])
    tcx = tsb.tile([128, 4], f32, tag="tcx", bufs=2)
    nc.scalar.activation(tcx[:], cxn[:], AF.Tanh)
    hxn = tsb.tile([128, 4], f32, tag="hxn", bufs=2)
    nc.vector.tensor_tensor(hxn[:], go[:], tcx[:], ALU.mult)
    nc.sync.dma_start(out=dout["o_hx"][:].rearrange("(j p) -> p j", p=128),
                      in_=hxn[:])

    # ================= q-head ============================================
    psum_q1 = ptail.tile([128, 1], f32, tag="ptail", bufs=1)
    q1T = w["q1T"].bitcast(f32)
    for k in range(4):
        nc.tensor.matmul(psum_q1[:], q1T[:, k, :], hxn[:, k:k + 1],
                         start=(k == 0), stop=(k == 3))
    z2 = tsb.tile([128, 1], f32, tag="z2", bufs=2)
    nc.scalar.activation(z2[:], psum_q1[:], AF.Relu, bias=w["q1b"].bitcast(f32))
    psum_q2 = ptail.tile([64, 1], f32, tag="ptail", bufs=1)
    nc.tensor.matmul(psum_q2[:], w["q2T"].bitcast(f32), z2[:], start=True, stop=True)
    z3 = tsb.tile([64, 1], f32, tag="z3", bufs=2)
    nc.scalar.activation(z3[:], psum_q2[:], AF.Relu, bias=w["q2b"].bitcast(f32))

    # ================= actor head + softmax ==============================
    psum_a1 = ptail.tile([16, 1], f32, tag="ptail", bufs=1)
    nc.tensor.matmul(psum_a1[:], w["aw1T"].bitcast(f32), z3[:], start=True, stop=True)
    a1 = tsb.tile([16, 1], f32, tag="a1", bufs=2)
    nc.scalar.activation(a1[:], psum_a1[:], AF.Relu, bias=w["ab1"].bitcast(f32))
    psum_al = ptail.tile([1, 5], f32, tag="ptail", bufs=1)
    nc.tensor.matmul(psum_al[:], a1[:], w["aw2T"].bitcast(f32), start=True, stop=True)
    lg = tsb.tile([1, 5], f32, tag="lg", bufs=2)
    nc.vector.tensor_tensor(lg[:], psum_al[:], w["ab2"].bitcast(f32), ALU.add)
    mx = tsb.tile([1, 1], f32, tag="mx", bufs=2)
    nc.vector.reduce_max(mx[:], lg[:], axis=AX.X)
    lgs = tsb.tile([1, 5], f32, tag="lgs", bufs=2)
    nc.vector.tensor_scalar(lgs[:], lg[:], mx[:], None, ALU.subtract)
    # exp via the already-loaded Sigmoid table: e^x = 1/(1/sigmoid(x) - 1)
    # (avoids a ~1.3us mid-kernel ACT function-set reload for Exp)
    sg = tsb.tile([1, 5], f32, tag="sg", bufs=2)
    nc.scalar.activation(sg[:], lgs[:], AF.Sigmoid)
    rsg = tsb.tile([1, 5], f32, tag="rsg", bufs=2)
    nc.vector.reciprocal(rsg[:], sg[:])
    rm1 = tsb.tile([1, 5], f32, tag="rm1", bufs=2)
    nc.vector.tensor_scalar(rm1[:], rsg[:], -1.0, None, ALU.add)
    ex = tsb.tile([1, 5], f32, tag="ex", bufs=2)
    nc.vector.reciprocal(ex[:], rm1[:])
    sm = tsb.tile([1, 1], f32, tag="sm", bufs=2)
    nc.vector.reduce_sum(sm[:], ex[:], axis=AX.X)
    rec = tsb.tile([1, 1], f32, tag="rec", bufs=2)
    nc.vector.reciprocal(rec[:], sm[:])
    probs = tsb.tile([1, 5], f32, tag="probs", bufs=2)
    nc.vector.tensor_scalar(probs[:], ex[:], rec[:], None, ALU.mult)
    nc.sync.dma_start(out=dout["o_actor"][:], in_=probs[:])

    # ================= critic heads ======================================
    cic = tsb.tile([1, 2], f32, tag="cic", bufs=2)
    for idx, (wn1, bn1, wn2, bn2) in enumerate((
            ("iw1T", "ib1", "iw2T", "ib2"),
            ("ew1T", "eb1", "ew2T", "eb2"))):
        psum_c1 = ptail.tile([16, 1], f32, tag="ptail", bufs=1)
        nc.tensor.matmul(psum_c1[:], w[wn1].bitcast(f32), z3[:], start=True, stop=True)
        c1 = tsb.tile([16, 1], f32, tag=f"c1h{idx}", bufs=2)
        nc.scalar.activation(c1[:], psum_c1[:], AF.Relu, bias=w[bn1].bitcast(f32))
        psum_c2 = ptail.tile([1, 1], f32, tag="ptail", bufs=1)
        nc.tensor.matmul(psum_c2[:], c1[:], w[wn2].bitcast(f32), start=True, stop=True)
        nc.vector.tensor_tensor(cic[:, idx:idx + 1], psum_c2[:],
                                w[bn2].bitcast(f32), ALU.add)
    nc.sync.dma_start(out=dout["o_cic"][:], in_=cic[:])


# ======================= host-side marshalling ===========================

def _lidar4(inputs, core):
    x = np.asarray(inputs["x"], dtype=np.float32)
    lidar = x[3:].reshape(2, N_POINTS)
    sh = lidar[:, core * NSH:(core + 1) * NSH]
    v = sh.reshape(2, G2, 2, F)
    return np.ascontiguousarray(v.transpose(2, 0, 1, 3).reshape(4, NSH // 2))


def _pack_weights(inputs):
    g = {k: np.asarray(v, dtype=np.float32) for k, v in inputs.items()}
    x = g["x"]

    def colmajor(vec, cols):
        return np.ascontiguousarray(vec.reshape(cols, 128).T)

    c1wg = np.zeros((4, 128), np.float32)
    c1wg[0:2, 0:64] = g["c1w"].T
    c1wg[2:4, 64:128] = g["c1w"].T
    pwT = np.zeros((384, 512), np.float32)
    pwT[0:288] = g["pw"].T

    def kmaj(wT, k):
        # (k*128, n) -> [128, k*n] laid out k-major to match the AP rearrange
        n = wT.shape[1]
        return np.ascontiguousarray(
            wT.reshape(k, 128, n).transpose(1, 0, 2).reshape(128, k * n))

    vals = {
        "c1wg": c1wg,
        "c1b2": np.concatenate([g["c1b"], g["c1b"]])[:, None],
        "c2wT": np.vstack([g["c2w"].T, g["c2w"].T]),
        "c2b": g["c2b"][:, None],
        "c3wT": g["c3w"].T,
        "c3b2": colmajor(g["c3b"], 2),
        "fcwT": kmaj(g["fcw"].T, 2),
        "fcb2": colmajor(g["fcb"], 2),
        "odo": x[:3][:, None],
        "ow1T": g["ow1"].T, "ob1": g["ob1"][:, None],
        "ow2T": g["ow2"].T, "ob2": g["ob2"][:, None],
        "pwT": kmaj(pwT, 3),
        "pb4": colmajor(g["pb"], 4),
        "bsum": colmajor(g["bih"] + g["bhh"], 16),
        "hxr": colmajor(g["hx"][0], 4),
        "cxr": colmajor(g["cx"][0], 4),
        "q1T": kmaj(g["q1w"].T, 4),
        "q1b": g["q1b"][:, None],
        "q2T": g["q2w"].T, "q2b": g["q2b"][:, None],
        "aw1T": g["aw1"].T, "ab1": g["ab1"][:, None],
        "aw2T": g["aw2"].T, "ab2": g["ab2"][None, :],
        "iw1T": g["iw1"].T, "ib1": g["ib1"][:, None],
        "iw2T": g["iw2"].T, "ib2": g["ib2"][None, :],
        "ew1T": g["ew1"].T, "eb1": g["eb1"][:, None],
        "ew2T": g["ew2"].T, "eb2": g["eb2"][None, :],
    }
    wpack = np.zeros((128, NCOL), np.float32)
    for nm, (c0, p, wd) in _COL_OFF.items():
        v = np.asarray(vals[nm], np.float32)
        assert v.shape == (p, wd), (nm, v.shape, (p, wd))
        wpack[0:p, c0:c0 + wd] = v

    def k4(wT):
        return np.ascontiguousarray(wT.reshape(4, 128, wT.shape[1]))

    return wpack, k4(g["wih"].T), k4(g["whh"].T)


def prep_in_maps(inputs):
    wpack, wihT, whhT = _pack_weights(inputs)
    maps = []
    for c in range(N_CORES):
        maps.append({
            "lidar4": _lidar4(inputs, c),
            "wpack": wpack,
            "wihT": wihT,
            "whhT": whhT,
        })
    return maps


# ======================= PJRT runner (cached jit) ========================

_CACHE = {}


def get_runner(reps: int = 1):
    """Build (or fetch cached) a callable running the NEFF on 8 cores."""
    if reps in _CACHE:
        return _CACHE[reps]
    import jax
    from jax.sharding import Mesh, PartitionSpec
    from jax.experimental.shard_map import shard_map
    from concourse.bass2jax import (_bass_exec_p, install_neuronx_cc_hook,
                                    partition_id_tensor)

    install_neuronx_cc_hook()
    nc = build(reps=reps)

    partition_name = (nc.partition_id_tensor.name
                      if nc.partition_id_tensor else None)
    in_names, out_names, out_avals, zero_outs = [], [], [], []
    for alloc in nc.m.functions[0].allocations:
        if not isinstance(alloc, mybir.MemoryLocationSet):
            continue
        name = alloc.memorylocations[0].name
        if alloc.kind == "ExternalInput":
            if name != partition_name:
                in_names.append(name)
        elif alloc.kind == "ExternalOutput":
            shape = tuple(alloc.tensor_shape)
            dtype = mybir.dt.np(alloc.dtype)
            out_names.append(name)
            out_avals.append(jax.core.ShapedArray(shape, dtype))
            zero_outs.append(np.zeros(shape, dtype))
    n_params = len(in_names)
    all_in_names = in_names + out_names + ([partition_name] if partition_name
                                           else [])

    def _body(*args):
        operands = list(args)
        if partition_name is not None:
            operands.append(partition_id_tensor())
        outs = _bass_exec_p.bind(
            *operands, out_avals=tuple(out_avals),
            in_names=tuple(all_in_names), out_names=tuple(out_names),
            lowering_input_output_aliases=(),
            sim_require_finite=False, sim_require_nnan=False, nc=nc)
        return tuple(outs)

    devices = jax.devices()[:N_CORES]
    mesh = Mesh(np.asarray(devices), ("core",))
    n_outs = len(out_names)
    sharded = jax.jit(
        shard_map(_body, mesh=mesh,
                  in_specs=(PartitionSpec("core"),) * (n_params + n_outs),
                  out_specs=(PartitionSpec("core"),) * n_outs,
                  check_rep=False),
        keep_unused=True)

    def stage_inputs(in_maps):
        concat_in = [
            np.concatenate([np.asarray(in_maps[c][name]) for c in range(N_CORES)],
                           axis=0)
            for name in in_names]
        concat_zeros = [np.zeros((N_CORES * z.shape[0], *z.shape[1:]), z.dtype)
                        for z in zero_outs]
        return [jax.device_put(a) for a in concat_in + concat_zeros]

    def run(in_maps, device_args=None):
        if device_args is None:
            device_args = stage_inputs(in_maps)
        out_arrs = sharded(*device_args)
        return [
            {name: np.asarray(out_arrs[i]).reshape(N_CORES, *out_avals[i].shape)[c]
             for i, name in enumerate(out_names)}
            for c in range(N_CORES)
        ]

    entry = {"run": run, "stage": stage_inputs, "sharded": sharded,
             "out_names": out_names, "out_avals": out_avals, "nc": nc}
    _CACHE[reps] = entry
    return entry


def kernel(**inputs):
    """Full-input entry point: returns the reference pytree."""
    in_maps = prep_in_maps(inputs)
    runner = get_runner(reps=1)
    results = runner["run"](in_maps)
    r = results[0]
    actor = r["o_actor"].astype(np.float32)
    ci = r["o_cic"][0:1].astype(np.float32)
    ce = r["o_cic"][1:2].astype(np.float32)
    hx_new = r["o_hx"].astype(np.float32).reshape(1, 512)
    cx_new = r["o_cx"].astype(np.float32).reshape(1, 512)
    ni = r["o_ni"].astype(np.float32)
    return (actor, ci, ce, (hx_new, cx_new), ni)


# revision 21
# speedup vs baseline: 4.4521x; 4.4521x over previous
"""Trainium2 Bass kernel for nn_ActorCritic (PointNet conv trunk + LSTM actor-critic).

Self-contained: hardcodes all shapes/sharding. Strategy:
  - Shard the N=262144 lidar points across 8 NeuronCores (32768 each).
  - Per core: conv trunk 2->64->128->256 as f32r PE matmuls over 512-col
    chunks (layer-1 packs 2 point-groups into K=4/M=128), relu+bias on ACT,
    per-chunk max-pool on DVE straight out of PSUM (bias+relu folded after
    pooling - both are monotonic per-channel).
  - AllGather the per-core (256,) channel maxes, reduce across cores locally.
  - Replicated tail on every core: fc -> pose-net concat -> pw -> LSTMCell ->
    actor softmax + twin critics. Matvecs keep M on partitions (N=1 matmuls)
    so the LSTM elementwise runs at [128,4] instead of [1,512].
  - All small weights travel in one packed [128, NCOL] tensor (2 DMAs) since
    per-dma_start fixed overhead (~0.65us) dominates small transfers.
  - Core 0's outputs are returned.
"""

import numpy as np

import concourse.bass as bass
import concourse.mybir as mybir
import concourse.tile as tile
from concourse import bacc

N_CORES = 8
N_POINTS = 262144
NSH = N_POINTS // N_CORES          # 32768 points per core
F = 512                            # free-dim chunk (psum bank)
G2 = NSH // (2 * F)                # 32 double-chunks (2 groups of 512 pts)

f32 = mybir.dt.float32
f32r = mybir.dt.float32r
AF = mybir.ActivationFunctionType
ALU = mybir.AluOpType
AX = mybir.AxisListType

# Packed small-weight layout: name -> (partition_rows, cols). Conv weights
# first so the first (small) DMA unblocks the conv trunk immediately.
_PACK_CONV = [
    ("c1wg", 4, 128), ("c1b2", 128, 1), ("c2wT", 128, 128), ("c2b", 128, 1),
    ("c3wT", 128, 256), ("c3b2", 128, 2),
]
_PACK_TAIL = [
    ("fcwT", 128, 2 * 256), ("fcb2", 128, 2),
    ("odo", 3, 1), ("ow1T", 3, 16), ("ob1", 16, 1),
    ("ow2T", 16, 32), ("ob2", 32, 1),
    ("pwT", 128, 3 * 512), ("pb4", 128, 4), ("bsum", 128, 16),
    ("hxr", 128, 4), ("cxr", 128, 4),
    ("q1T", 128, 4 * 128), ("q1b", 128, 1), ("q2T", 128, 64), ("q2b", 64, 1),
    ("aw1T", 64, 16), ("ab1", 16, 1), ("aw2T", 16, 5), ("ab2", 1, 5),
    ("iw1T", 64, 16), ("ib1", 16, 1), ("iw2T", 16, 1), ("ib2", 1, 1),
    ("ew1T", 64, 16), ("eb1", 16, 1), ("ew2T", 16, 1), ("eb2", 1, 1),
]
_PACK = _PACK_CONV + _PACK_TAIL
_COL_OFF = {}
_c = 0
for _nm, _p, _w in _PACK:
    _COL_OFF[_nm] = (_c, _p, _w)
    _c += _w
NCOL = _c
CONV_COLS = sum(w for _, _, w in _PACK_CONV)

_INPUT_DECLS = {
    "lidar4": ((4, NSH // 2), f32r),
    "wpack": ((128, NCOL), f32r),
    "wihT": ((4, 128, 2048), f32),
    "whhT": ((4, 128, 2048), f32),
}

_OUTPUT_DECLS = {
    "o_actor": (5,),
    "o_cic": (2,),
    "o_hx": (512,),
    "o_cx": (512,),
    "o_ni": (288,),
}


class _WSlices:
    """Weight APs sliced out of the packed SBUF tile."""

    def __init__(self, wpack_sb):
        self._t = wpack_sb

    def __getitem__(self, name):
        c0, p, wd = _COL_OFF[name]
        ap = self._t[0:p, c0:c0 + wd]
        if name == "fcwT":
            return ap.rearrange("p (k n) -> p k n", k=2)
        if name == "pwT":
            return ap.rearrange("p (k n) -> p k n", k=3)
        if name == "q1T":
            return ap.rearrange("p (k n) -> p k n", k=4)
        return ap


def build(reps: int = 1, sim_mode: bool = False, part: str = 'full'):
    nc = bacc.Bacc("TRN2", target_bir_lowering=False, debug=False,
                   num_devices=1 if sim_mode else N_CORES)
    din = {name: nc.dram_tensor(name, list(shape), dt, kind="ExternalInput").ap()
           for name, (shape, dt) in _INPUT_DECLS.items()}
    dout = {name: nc.dram_tensor(name, list(shape), f32, kind="ExternalOutput").ap()
            for name, shape in _OUTPUT_DECLS.items()}

    with tile.TileContext(nc) as tc:
        with tc.tile_pool(name="wsb", bufs=1) as wsb, \
             tc.tile_pool(name="csb", bufs=1) as csb, \
             tc.tile_pool(name="tsb", bufs=1) as tsb, \
             tc.tile_pool(name="pconv", bufs=1, space="PSUM") as pconv, \
             tc.tile_pool(name="ptail", bufs=1, space="PSUM") as ptail, \
             tc.tile_pool(name="dpool", bufs=1, space="DRAM") as dpool:

            actwarm = wsb.tile([1, 1], f32, name="actwarm")
            nc.vector.memset(actwarm[:], 0.0)
            actwarm2 = wsb.tile([1, 1], f32, name="actwarm2")
            nc.scalar.activation(actwarm2[:], actwarm[:], AF.Sigmoid)

            wpack_sb = wsb.tile([128, NCOL], f32r, name="wpack_sb")
            # conv weights first (small DMA), rest of the small weights second
            nc.sync.dma_start(out=wpack_sb[:, 0:CONV_COLS],
                              in_=din["wpack"][:, 0:CONV_COLS])
            w = _WSlices(wpack_sb)

            big = {}
            for name in ("wihT", "whhT"):
                big[name] = wsb.tile([128, 4, 2048], f32, name=f"w_{name}")

            def load_big_weights():
                nc.sync.dma_start(out=wpack_sb[:, CONV_COLS:NCOL],
                                  in_=din["wpack"][:, CONV_COLS:NCOL])
                for name in ("wihT", "whhT"):
                    t = big[name]
                    ns = 2048 // 4
                    for j in range(4):
                        nc.sync.dma_start(
                            out=t[:, :, j * ns:(j + 1) * ns],
                            in_=din[name][:, :, j * ns:(j + 1) * ns]
                            .rearrange("k p n -> p k n"))

            for i in range(reps):
                _one_rep(nc, tc, din, dout, w, big, csb, tsb, pconv, ptail,
                         dpool, sim_mode=sim_mode,
                         post_lidar=load_big_weights if i == 0 else None,
                         part=part)

    nc.compile()
    return nc


def _one_rep(nc, tc, din, dout, w, big, csb, tsb, pconv, ptail, dpool,
             sim_mode=False, post_lidar=None, part="full"):
    # ================= conv trunk over point chunks =======================
    lidar_sb = csb.tile([4, NSH // 2], f32r, tag="lidar", bufs=1)
    nblk = 4
    bs = (NSH // 2) // nblk
    for j in range(nblk):
        nc.sync.dma_start(out=lidar_sb[:, j * bs:(j + 1) * bs],
                          in_=din["lidar4"][:, j * bs:(j + 1) * bs])
    if post_lidar is not None:
        post_lidar()

    cm0 = csb.tile([128, G2], f32, tag="cm0", bufs=2)
    cm1 = csb.tile([128, G2], f32, tag="cm1", bufs=2)

    for g in range(G2):
        gs = slice(g * F, (g + 1) * F)
        p1 = pconv.tile([128, F], f32, tag="p1", bufs=1)
        nc.tensor.matmul(p1[:], w["c1wg"], lidar_sb[:, gs], start=True, stop=True)
        h1 = csb.tile([128, F], f32r, tag="h1", bufs=3)
        nc.scalar.activation(h1[:], p1[:], AF.Relu, bias=w["c1b2"].bitcast(f32))

        p2a = pconv.tile([128, F], f32, tag="p2", bufs=2)
        nc.tensor.matmul(p2a[:], w["c2wT"][0:64, :], h1[0:64, :], start=True, stop=True)
        p2b = pconv.tile([128, F], f32, tag="p2", bufs=2)
        nc.tensor.matmul(p2b[:], w["c2wT"][64:128, :], h1[64:128, :], start=True, stop=True)
        h2 = csb.tile([128, 2 * F], f32r, tag="h2", bufs=2)
        nc.scalar.activation(h2[:, 0:F], p2a[:], AF.Relu, bias=w["c2b"].bitcast(f32))
        nc.scalar.activation(h2[:, F:2 * F], p2b[:], AF.Relu, bias=w["c2b"].bitcast(f32))
        h2a, h2b = h2[:, 0:F], h2[:, F:2 * F]

        # both groups' L3 outputs for one m-tile share a [128, 1024] psum
        # (2 banks) so the max-pool is a single DVE reduce per m-tile.
        p3m0 = pconv.tile([128, 2 * F], f32, tag="p3", bufs=2)
        nc.tensor.matmul(p3m0[:, 0:F], w["c3wT"][:, 0:128], h2a, start=True, stop=True)
        nc.tensor.matmul(p3m0[:, F:2 * F], w["c3wT"][:, 0:128], h2b, start=True, stop=True)
        nc.vector.reduce_max(cm0[:, g:g + 1], p3m0[:], axis=AX.X)
        p3m1 = pconv.tile([128, 2 * F], f32, tag="p3", bufs=2)
        nc.tensor.matmul(p3m1[:, 0:F], w["c3wT"][:, 128:256], h2a, start=True, stop=True)
        nc.tensor.matmul(p3m1[:, F:2 * F], w["c3wT"][:, 128:256], h2b, start=True, stop=True)
        nc.vector.reduce_max(cm1[:, g:g + 1], p3m1[:], axis=AX.X)

        if g == 20:
            # LSTM whh-part: fills PE bubbles while conv runs; fold to SBUF
            # immediately so the psum bank frees for the tail chain.
            psum_gh = ptail.tile([128, 16], f32, tag="ptail", bufs=1)
            for j in range(16):
                js = slice(j * 128, (j + 1) * 128)
                for k in range(4):
                    nc.tensor.matmul(psum_gh[:, j:j + 1], big["whhT"][:, k, js],
                                     w["hxr"].bitcast(f32)[:, k:k + 1],
                                     start=(k == 0), stop=(k == 3))
            whhb = tsb.tile([128, 16], f32, tag="whhb", bufs=2)
            nc.vector.tensor_tensor(whhb[:], psum_gh[:], w["bsum"].bitcast(f32),
                                    ALU.add)

    praw = tsb.tile([128, 2], f32, tag="praw", bufs=2)
    nc.vector.reduce_max(praw[:, 0:1], cm0[:], axis=AX.X)
    nc.vector.reduce_max(praw[:, 1:2], cm1[:], axis=AX.X)
    if part == "conv":
        nc.sync.dma_start(out=dout["o_ni"][0:256].rearrange("(t p) -> p t", p=128),
                          in_=praw[:])
        return

    # ================= AllGather of per-core channel maxes ================
    ag_in = dpool.tile([256], f32, tag="ag_in", bufs=2)
    ag_out = dpool.tile([N_CORES * 256], f32, tag="ag_out", bufs=2,
                        addr_space="Local" if sim_mode else "Shared")
    nc.sync.dma_start(out=ag_in[:].rearrange("(t p) -> p t", p=128), in_=praw[:])
    if sim_mode:
        for r in range(N_CORES):
            nc.sync.dma_start(out=ag_out[r * 256:(r + 1) * 256], in_=ag_in[:])
    else:
        nc.gpsimd.collective_compute(
            "AllGather", ALU.bypass,
            replica_groups=[list(range(N_CORES))],
            ins=[ag_in[:].opt()], outs=[ag_out[:].opt()])
    agt = tsb.tile([128, 16], f32, tag="agt", bufs=2)
    nc.sync.dma_start(out=agt[:],
                      in_=ag_out[:].rearrange("(r t p) -> p r t", p=128, t=2))
    pm = tsb.tile([128, 2], f32, tag="pm", bufs=2)
    nc.vector.reduce_max(pm[:], agt[:].rearrange("p (r t) -> p t r", r=8, t=2),
                         axis=AX.X)
    hmax = tsb.tile([128, 2], f32, tag="hmax", bufs=2)
    for t in range(2):
        nc.scalar.activation(hmax[:, t:t + 1], pm[:, t:t + 1], AF.Relu,
                             bias=w["c3b2"].bitcast(f32)[:, t:t + 1])

    # ================= fc: obs = fcw @ pooled + fcb (no relu) =============
    psum_obs = ptail.tile([128, 2], f32, tag="ptail", bufs=1)
    for m in range(2):
        ms = slice(m * 128, (m + 1) * 128)
        for k in range(2):
            nc.tensor.matmul(psum_obs[:, m:m + 1],
                             w["fcwT"].bitcast(f32)[:, k, ms],
                             hmax[:, k:k + 1], start=(k == 0), stop=(k == 1))
    ni01 = tsb.tile([128, 2], f32, tag="ni01", bufs=2)
    for m in range(2):
        nc.scalar.activation(ni01[:, m:m + 1], psum_obs[:, m:m + 1], AF.Identity,
                             bias=w["fcb2"].bitcast(f32)[:, m:m + 1])
    nc.sync.dma_start(out=dout["o_ni"][0:256].rearrange("(m p) -> p m", p=128),
                      in_=ni01[:])

    # ================= pose net ==========================================
    pp1 = ptail.tile([16, 1], f32, tag="ptail", bufs=1)
    nc.tensor.matmul(pp1[:], w["ow1T"].bitcast(f32), w["odo"].bitcast(f32),
                     start=True, stop=True)
    p1s = tsb.tile([16, 1], f32, tag="p1s", bufs=2)
    nc.scalar.activation(p1s[:], pp1[:], AF.Sigmoid, bias=w["ob1"].bitcast(f32))
    pp2 = ptail.tile([32, 1], f32, tag="ptail", bufs=1)
    nc.tensor.matmul(pp2[:], w["ow2T"].bitcast(f32), p1s[:], start=True, stop=True)
    ni2 = tsb.tile([32, 1], f32, tag="ni2", bufs=2)
    nc.scalar.activation(ni2[:], pp2[:], AF.Relu, bias=w["ob2"].bitcast(f32))
    nc.sync.dma_start(out=dout["o_ni"][256:288], in_=ni2[:])

    # ================= pw: z = relu(pw @ net_input + pb) ==================
    psum_z = ptail.tile([128, 4], f32, tag="ptail", bufs=1)
    pwT = w["pwT"].bitcast(f32)
    for m in range(4):
        ms = slice(m * 128, (m + 1) * 128)
        nc.tensor.matmul(psum_z[:, m:m + 1], pwT[:, 0, ms], ni01[:, 0:1],
                         start=True, stop=False)
        nc.tensor.matmul(psum_z[:, m:m + 1], pwT[:, 1, ms], ni01[:, 1:2],
                         start=False, stop=False)
        nc.tensor.matmul(psum_z[:, m:m + 1], pwT[0:32, 2, ms], ni2[:],
                         start=False, stop=True)
    z_sb = tsb.tile([128, 4], f32, tag="z_sb", bufs=2)
    for m in range(4):
        nc.scalar.activation(z_sb[:, m:m + 1], psum_z[:, m:m + 1], AF.Relu,
                             bias=w["pb4"].bitcast(f32)[:, m:m + 1])

    # ================= LSTM cell =========================================
    psum_g = ptail.tile([128, 16], f32, tag="ptail", bufs=1)
    for j in range(16):
        js = slice(j * 128, (j + 1) * 128)
        for k in range(4):
            nc.tensor.matmul(psum_g[:, j:j + 1], big["wihT"][:, k, js],
                             z_sb[:, k:k + 1], start=(k == 0), stop=(k == 3))
    gates = tsb.tile([128, 16], f32, tag="gates", bufs=2)
    nc.vector.tensor_tensor(gates[:], psum_g[:], whhb[:], ALU.add)
    gi = tsb.tile([128, 4], f32, tag="gi", bufs=2)
    gf = tsb.tile([128, 4], f32, tag="gf", bufs=2)
    gg = tsb.tile([128, 4], f32, tag="gg", bufs=2)
    go = tsb.tile([128, 4], f32, tag="go", bufs=2)
    nc.scalar.activation(gi[:], gates[:, 0:4], AF.Sigmoid)
    nc.scalar.activation(gf[:], gates[:, 4:8], AF.Sigmoid)
    nc.scalar.activation(gg[:], gates[:, 8:12], AF.Tanh)
    nc.scalar.activation(go[:], gates[:, 12:16], AF.Sigmoid)
    t1 = tsb.tile([128, 4], f32, tag="t1", bufs=2)
    nc.vector.tensor_tensor(t1[:], gi[:], gg[:], ALU.mult)
    t2 = tsb.tile([128, 4], f32, tag="t2", bufs=2)
    nc.vector.tensor_tensor(t2[:], gf[:], w["cxr"].bitcast(f32), ALU.mult)
    cxn = tsb.tile([128, 4], f32, tag="cxn", bufs=2)
    nc.vector.tensor_tensor(cxn[:], t1[:], t2[:], ALU.add)
    nc.sync.dma_start(out=dout["o_cx"][:].rearrange("(j p) -> p j", p=128),
                      in_=cxn[:])
    tcx = tsb.tile([128, 4], f32, tag="tcx", bufs=2)
    nc.scalar.activation(tcx[:], cxn[:], AF.Tanh)
    hxn = tsb.tile([128, 4], f32, tag="hxn", bufs=2)
    nc.vector.tensor_tensor(hxn[:], go[:], tcx[:], ALU.mult)
    nc.sync.dma_start(out=dout["o_hx"][:].rearrange("(j p) -> p j", p=128),
                      in_=hxn[:])

    # ================= q-head ============================================
    psum_q1 = ptail.tile([128, 1], f32, tag="ptail", bufs=1)
    q1T = w["q1T"].bitcast(f32)
    for k in range(4):
        nc.tensor.matmul(psum_q1[:], q1T[:, k, :], hxn[:, k:k + 1],
                         start=(k == 0), stop=(k == 3))
    z2 = tsb.tile([128, 1], f32, tag="z2", bufs=2)
    nc.scalar.activation(z2[:], psum_q1[:], AF.Relu, bias=w["q1b"].bitcast(f32))
    psum_q2 = ptail.tile([64, 1], f32, tag="ptail", bufs=1)
    nc.tensor.matmul(psum_q2[:], w["q2T"].bitcast(f32), z2[:], start=True, stop=True)
    z3 = tsb.tile([64, 1], f32, tag="z3", bufs=2)
    nc.scalar.activation(z3[:], psum_q2[:], AF.Relu, bias=w["q2b"].bitcast(f32))

    # ================= actor head + softmax ==============================
    psum_a1 = ptail.tile([16, 1], f32, tag="ptail", bufs=1)
    nc.tensor.matmul(psum_a1[:], w["aw1T"].bitcast(f32), z3[:], start=True, stop=True)
    a1 = tsb.tile([16, 1], f32, tag="a1", bufs=2)
    nc.scalar.activation(a1[:], psum_a1[:], AF.Relu, bias=w["ab1"].bitcast(f32))
    psum_al = ptail.tile([1, 5], f32, tag="ptail", bufs=1)
    nc.tensor.matmul(psum_al[:], a1[:], w["aw2T"].bitcast(f32), start=True, stop=True)
    lg = tsb.tile([1, 5], f32, tag="lg", bufs=2)
    nc.vector.tensor_tensor(lg[:], psum_al[:], w["ab2"].bitcast(f32), ALU.add)
    mx = tsb.tile([1, 1], f32, tag="mx", bufs=2)
    nc.vector.reduce_max(mx[:], lg[:], axis=AX.X)
    lgs = tsb.tile([1, 5], f32, tag="lgs", bufs=2)
    nc.vector.tensor_scalar(lgs[:], lg[:], mx[:], None, ALU.subtract)
    # exp via the already-loaded Sigmoid table: e^x = 1/(1/sigmoid(x) - 1)
    # (avoids a ~1.3us mid-kernel ACT function-set reload for Exp)
    sg = tsb.tile([1, 5], f32, tag="sg", bufs=2)
    nc.scalar.activation(sg[:], lgs[:], AF.Sigmoid)
    rsg = tsb.tile([1, 5], f32, tag="rsg", bufs=2)
    nc.vector.reciprocal(rsg[:], sg[:])
    rm1 = tsb.tile([1, 5], f32, tag="rm1", bufs=2)
    nc.vector.tensor_scalar(rm1[:], rsg[:], -1.0, None, ALU.add)
    ex = tsb.tile([1, 5], f32, tag="ex", bufs=2)
    nc.vector.reciprocal(ex[:], rm1[:])
    sm = tsb.tile([1, 1], f32, tag="sm", bufs=2)
    nc.vector.reduce_sum(sm[:], ex[:], axis=AX.X)
    rec = tsb.tile([1, 1], f32, tag="rec", bufs=2)
    nc.vector.reciprocal(rec[:], sm[:])
    probs = tsb.tile([1, 5], f32, tag="probs", bufs=2)
    nc.vector.tensor_scalar(probs[:], ex[:], rec[:], None, ALU.mult)
    nc.sync.dma_start(out=dout["o_actor"][:], in_=probs[:])

    # ================= critic heads ======================================
    cic = tsb.tile([1, 2], f32, tag="cic", bufs=2)
    for idx, (wn1, bn1, wn2, bn2) in enumerate((
            ("iw1T", "ib1", "iw2T", "ib2"),
            ("ew1T", "eb1", "ew2T", "eb2"))):
        psum_c1 = ptail.tile([16, 1], f32, tag="ptail", bufs=1)
        nc.tensor.matmul(psum_c1[:], w[wn1].bitcast(f32), z3[:], start=True, stop=True)
        c1 = tsb.tile([16, 1], f32, tag=f"c1h{idx}", bufs=2)
        nc.scalar.activation(c1[:], psum_c1[:], AF.Relu, bias=w[bn1].bitcast(f32))
        psum_c2 = ptail.tile([1, 1], f32, tag="ptail", bufs=1)
        nc.tensor.matmul(psum_c2[:], c1[:], w[wn2].bitcast(f32), start=True, stop=True)
        nc.vector.tensor_tensor(cic[:, idx:idx + 1], psum_c2[:],
                                w[bn2].bitcast(f32), ALU.add)
    nc.sync.dma_start(out=dout["o_cic"][:], in_=cic[:])


# ======================= host-side marshalling ===========================

def _lidar4(inputs, core):
    x = np.asarray(inputs["x"], dtype=np.float32)
    lidar = x[3:].reshape(2, N_POINTS)
    sh = lidar[:, core * NSH:(core + 1) * NSH]
    v = sh.reshape(2, G2, 2, F)
    return np.ascontiguousarray(v.transpose(2, 0, 1, 3).reshape(4, NSH // 2))


def _pack_weights(inputs):
    g = {k: np.asarray(v, dtype=np.float32) for k, v in inputs.items()}
    x = g["x"]

    def colmajor(vec, cols):
        return np.ascontiguousarray(vec.reshape(cols, 128).T)

    c1wg = np.zeros((4, 128), np.float32)
    c1wg[0:2, 0:64] = g["c1w"].T
    c1wg[2:4, 64:128] = g["c1w"].T
    pwT = np.zeros((384, 512), np.float32)
    pwT[0:288] = g["pw"].T

    def kmaj(wT, k):
        # (k*128, n) -> [128, k*n] laid out k-major to match the AP rearrange
        n = wT.shape[1]
        return np.ascontiguousarray(
            wT.reshape(k, 128, n).transpose(1, 0, 2).reshape(128, k * n))

    vals = {
        "c1wg": c1wg,
        "c1b2": np.concatenate([g["c1b"], g["c1b"]])[:, None],
        "c2wT": np.vstack([g["c2w"].T, g["c2w"].T]),
        "c2b": g["c2b"][:, None],
        "c3wT": g["c3w"].T,
        "c3b2": colmajor(g["c3b"], 2),
        "fcwT": kmaj(g["fcw"].T, 2),
        "fcb2": colmajor(g["fcb"], 2),
        "odo": x[:3][:, None],
        "ow1T": g["ow1"].T, "ob1": g["ob1"][:, None],
        "ow2T": g["ow2"].T, "ob2": g["ob2"][:, None],
        "pwT": kmaj(pwT, 3),
        "pb4": colmajor(g["pb"], 4),
        "bsum": colmajor(g["bih"] + g["bhh"], 16),
        "hxr": colmajor(g["hx"][0], 4),
        "cxr": colmajor(g["cx"][0], 4),
        "q1T": kmaj(g["q1w"].T, 4),
        "q1b": g["q1b"][:, None],
        "q2T": g["q2w"].T, "q2b": g["q2b"][:, None],
        "aw1T": g["aw1"].T, "ab1": g["ab1"][:, None],
        "aw2T": g["aw2"].T, "ab2": g["ab2"][None, :],
        "iw1T": g["iw1"].T, "ib1": g["ib1"][:, None],
        "iw2T": g["iw2"].T, "ib2": g["ib2"][None, :],
        "ew1T": g["ew1"].T, "eb1": g["eb1"][:, None],
        "ew2T": g["ew2"].T, "eb2": g["eb2"][None, :],
    }
    wpack = np.zeros((128, NCOL), np.float32)
    for nm, (c0, p, wd) in _COL_OFF.items():
        v = np.asarray(vals[nm], np.float32)
        assert v.shape == (p, wd), (nm, v.shape, (p, wd))
        wpack[0:p, c0:c0 + wd] = v

    def k4(wT):
        return np.ascontiguousarray(wT.reshape(4, 128, wT.shape[1]))

    return wpack, k4(g["wih"].T), k4(g["whh"].T)


def prep_in_maps(inputs):
    wpack, wihT, whhT = _pack_weights(inputs)
    maps = []
    for c in range(N_CORES):
        maps.append({
            "lidar4": _lidar4(inputs, c),
            "wpack": wpack,
            "wihT": wihT,
            "whhT": whhT,
        })
    return maps


# ======================= PJRT runner (cached jit) ========================

_CACHE = {}


def get_runner(reps: int = 1, part: str = "full"):
    """Build (or fetch cached) a callable running the NEFF on 8 cores."""
    key = (reps, part)
    if key in _CACHE:
        return _CACHE[key]
    import jax
    from jax.sharding import Mesh, PartitionSpec
    from jax.experimental.shard_map import shard_map
    from concourse.bass2jax import (_bass_exec_p, install_neuronx_cc_hook,
                                    partition_id_tensor)

    install_neuronx_cc_hook()
    nc = build(reps=reps, part=part)

    partition_name = (nc.partition_id_tensor.name
                      if nc.partition_id_tensor else None)
    in_names, out_names, out_avals, zero_outs = [], [], [], []
    for alloc in nc.m.functions[0].allocations:
        if not isinstance(alloc, mybir.MemoryLocationSet):
            continue
        name = alloc.memorylocations[0].name
        if alloc.kind == "ExternalInput":
            if name != partition_name:
                in_names.append(name)
        elif alloc.kind == "ExternalOutput":
            shape = tuple(alloc.tensor_shape)
            dtype = mybir.dt.np(alloc.dtype)
            out_names.append(name)
            out_avals.append(jax.core.ShapedArray(shape, dtype))
            zero_outs.append(np.zeros(shape, dtype))
    n_params = len(in_names)
    all_in_names = in_names + out_names + ([partition_name] if partition_name
                                           else [])

    def _body(*args):
        operands = list(args)
        if partition_name is not None:
            operands.append(partition_id_tensor())
        outs = _bass_exec_p.bind(
            *operands, out_avals=tuple(out_avals),
            in_names=tuple(all_in_names), out_names=tuple(out_names),
            lowering_input_output_aliases=(),
            sim_require_finite=False, sim_require_nnan=False, nc=nc)
        return tuple(outs)

    devices = jax.devices()[:N_CORES]
    mesh = Mesh(np.asarray(devices), ("core",))
    n_outs = len(out_names)
    sharded = jax.jit(
        shard_map(_body, mesh=mesh,
                  in_specs=(PartitionSpec("core"),) * (n_params + n_outs),
                  out_specs=(PartitionSpec("core"),) * n_outs,
                  check_rep=False),
        keep_unused=True)

    def stage_inputs(in_maps):
        concat_in = [
            np.concatenate([np.asarray(in_maps[c][name]) for c in range(N_CORES)],
                           axis=0)
            for name in in_names]
        concat_zeros = [np.zeros((N_CORES * z.shape[0], *z.shape[1:]), z.dtype)
                        for z in zero_outs]
        return [jax.device_put(a) for a in concat_in + concat_zeros]

    def run(in_maps, device_args=None):
        if device_args is None:
            device_args = stage_inputs(in_maps)
        out_arrs = sharded(*device_args)
        return [
            {name: np.asarray(out_arrs[i]).reshape(N_CORES, *out_avals[i].shape)[c]
             for i, name in enumerate(out_names)}
            for c in range(N_CORES)
        ]

    entry = {"run": run, "stage": stage_inputs, "sharded": sharded,
             "out_names": out_names, "out_avals": out_avals, "nc": nc}
    _CACHE[key] = entry
    return entry


def kernel(**inputs):
    """Full-input entry point: returns the reference pytree."""
    in_maps = prep_in_maps(inputs)
    runner = get_runner(reps=1)
    results = runner["run"](in_maps)
    r = results[0]
    actor = r["o_actor"].astype(np.float32)
    ci = r["o_cic"][0:1].astype(np.float32)
    ce = r["o_cic"][1:2].astype(np.float32)
    hx_new = r["o_hx"].astype(np.float32).reshape(1, 512)
    cx_new = r["o_cx"].astype(np.float32).reshape(1, 512)
    ni = r["o_ni"].astype(np.float32)
    return (actor, ci, ce, (hx_new, cx_new), ni)
